# revision 14
# baseline (speedup 1.0000x reference)
"""Trainium2 Bass kernel for a YOLO-style detection loss.

Sharding: data-parallel over batch — 8 NeuronCores, 4 batches/core.
Per-core partial sums land in a [128, 7] tile; the host sums the 8
tiles and assembles the 4 scalar losses (replacing the all-reduce of
4 scalars).

The loss touches pred densely only through the objectness channel
(BCE vs 0 over every cell); the class/box terms need the 85 logits at
the <=2048 assigned cells.  The host routes data (extracts channel 4,
gathers the 85-float rows per target, precomputes target-derived
constants: grid offsets, small_weight, dedup flags) — all pure data
movement/indexing, as in the indirect-DMA version, but the gather now
happens host-side so the device never pays the serialized GpSimd
indirect-DMA issue + latency chain.  All loss arithmetic on pred
values runs on device:

1. OBJ stream: softplus over channel 4 of every cell (one [128, 263]
   bf16 tile): Exp pass then per-scale Ln(1+x) passes whose accum_out
   gives the per-scale column sums directly.
2. TGT stream: one [128, 276] bf16 tile holding 3 groups x 85 gathered
   logits + per-target constants.  Box decode uses Exp(scale=-1) +
   add/reciprocal for sigmoid, Exp(min(x,4)) for wh; the weighted sums
   come out of scalar_tensor_tensor accum_out.  Class softplus sum
   falls out of the Ln pass's accum_out; the target-class logit
   correction is a host-gathered column the device just sums.

softplus(x) = ln(exp(x) + 1); Exp/Ln are pinned to the single ACT
table that holds both (natural_log_exp_and_others) to avoid
per-instruction table reloads.  bf16 tiles: all accumulations land in
fp32 (accum_out / reduce dst); tolerance is 2e-2, bf16 input rounding
contributes ~1e-4.
"""

import numpy as np
import ml_dtypes

from concourse import bass, bacc, mybir
from concourse import bass_utils
from concourse.tile import TileContext

F32 = mybir.dt.float32
BF16 = mybir.dt.bfloat16
BF16_NP = ml_dtypes.bfloat16

NUM_CLASSES = 80
STAL_GAMMA = np.float32(2.0)
BATCH = 32
NCORES = 8
BPC = BATCH // NCORES          # batches per core
CH = 5 + NUM_CLASSES
HW = (80 * 80, 40 * 40, 20 * 20)
WS = (80, 40, 20)
# OBJ stream: per-scale column blocks, scale 2 padded to 128*13
OBJ_COLS = (HW[0] * BPC // 128, HW[1] * BPC // 128, 1664 // 128)  # 200,50,13
GROUPS = 3                                  # 128 targets each
TPAD = 128 * GROUPS                         # 384; mean load is ~256/core
PAD_VAL = np.float32(-30.0)                 # neutral logit for padding rows
# TGT tile column layout (bf16), GROUPS-interleaved like the VA rows
TC_VA = 0                                   # 3 x 85 gathered logits
TC_SUB = GROUPS * CH                        # 255: w*cx-gx etc, 3 x 4
TC_SWM = TC_SUB + GROUPS * 4                # 267: sw/4/w              3
TC_WOB = TC_SWM + GROUPS                    # 270: dedup/(B*HW_s)      3
TC_COR = TC_WOB + GROUPS                    # 273: target-class logit  3
NTGT = TC_COR + GROUPS                      # 276
# output partial tile column layout
OC_WSP = 0      # class softplus-sum term
OC_OBJ = 1      # 3 cols: per-scale objectness softplus sums
OC_BOX = 4
OC_POS = 5      # objectness positive-cell correction (pre-scaled)
OC_CORR = 6
NOUT = 7

_NC_CACHE = None


def _single_act_table(arch):
    """Empty out every activation table except natural_log_exp_and_others
    (which holds all the functions this kernel uses), so the table-load
    pass can only ever pick that one table -> exactly one ACT_TABLE_LOAD
    instead of a conservative extra load of table 0."""
    tabs = _ORIG_TABLES(arch)
    out = {}
    for name, fns in tabs.items():
        out[name] = fns if name == "natural_log_exp_and_others" \
            else type(fns)()
    return out


_ORIG_TABLES = bacc.get_activation_tables


def _build_nc():
    nc = bacc.Bacc("TRN2", target_bir_lowering=False, debug=False)
    obj_t = nc.dram_tensor("OBJ", [128, sum(OBJ_COLS)], BF16,
                           kind="ExternalInput")
    tgt_t = nc.dram_tensor("TGT", [128, NTGT], BF16, kind="ExternalInput")
    out_t = nc.dram_tensor("OUT", [128, NOUT], F32, kind="ExternalOutput")

    EXP = mybir.ActivationFunctionType.Exp
    LN = mybir.ActivationFunctionType.Ln
    AX = mybir.AxisListType
    ALU = mybir.AluOpType
    NOB = sum(OBJ_COLS)
    with nc.allow_low_precision("bf16 validated: tolerance 2e-2, "
                                "bf16 rounding contributes ~1e-4"), \
            TileContext(nc) as tc:
        with tc.tile_pool(name="persist", bufs=1) as pp:
            part = pp.tile([128, NOUT], F32)
            tg = pp.tile([128, NTGT], BF16)
            ob = pp.tile([128, NOB], BF16)
            l1 = pp.tile([128, GROUPS], BF16)
            g3 = pp.tile([128, GROUPS], BF16)
            sc = pp.tile([128, GROUPS], BF16)

            # TGT on the scalar HWDGE ring, OBJ on the sync ring (the
            # only two hardware DGE rings); OUT reuses the warm scalar
            # ring at the end
            nc.scalar.dma_start(out=tg[:], in_=tgt_t.ap())
            nc.sync.dma_start(out=ob[:], in_=obj_t.ap())

            v3 = tg[:, :TC_SUB].rearrange("p (j c) -> p j c", c=CH)
            sub3 = tg[:, TC_SUB:TC_SWM].rearrange("p (j c) -> p j c", c=4)

            # ---- per-target math ----
            # negate-and-clamp wh so ONE Exp(scale=-1) pass decodes all
            # four box channels: exp(-(-min(x,4))) == exp(min(x,4))
            nc.vector.tensor_scalar(v3[:, :, 2:4], v3[:, :, 2:4], -1.0, -4.0,
                                    op0=ALU.mult, op1=ALU.max)
            nc.scalar.activation(v3[:, :, 0:4], v3[:, :, 0:4], EXP,
                                 scale=-1.0)
            # sigmoid = 1/(1+exp(-x)) via DVE reciprocal
            nc.vector.tensor_scalar_add(v3[:, :, 0:2], v3[:, :, 0:2], 1.0)
            nc.vector.reciprocal(v3[:, :, 0:2], v3[:, :, 0:2])
            # objectness positive-cell correction (raw channel 4),
            # per-scale 1/(B*HW_s) prefolded into the WOB column
            nc.vector.scalar_tensor_tensor(
                sc[:], v3[:, :, 4], 0.0, tg[:, TC_WOB:TC_WOB + GROUPS],
                op0=ALU.bypass, op1=ALU.mult,
                accum_out=part[:, OC_POS:OC_POS + 1])
            # class-logit correction: host-gathered column, just sum it
            nc.vector.reduce_sum(part[:, OC_CORR:OC_CORR + 1],
                                 tg[:, TC_COR:TC_COR + GROUPS], axis=AX.X)
            # ---- dense objectness stream (ACT engine) ----
            nc.scalar.activation(ob[:], ob[:], EXP)
            nc.scalar.activation(ob[:], ob[:], LN, bias=1.0)
            ocol = 0
            for s in range(3):
                w = OBJ_COLS[s]
                nc.vector.reduce_sum(part[:, OC_OBJ + s:OC_OBJ + s + 1],
                                     ob[:, ocol:ocol + w], axis=AX.X)
                ocol += w
            # l1 in grid units: |dec - (w*tgt - g)|; 1/w folded into SWM
            nc.vector.tensor_sub(v3[:, :, 0:4], v3[:, :, 0:4], sub3)
            nc.vector.reduce_sum(l1[:], v3[:, :, 0:4], axis=AX.X,
                                 apply_absolute_value=True)
            nc.vector.scalar_tensor_tensor(
                g3[:], l1[:], 0.0, tg[:, TC_SWM:TC_SWM + GROUPS],
                op0=ALU.bypass, op1=ALU.mult,
                accum_out=part[:, OC_BOX:OC_BOX + 1])
            # class softplus sum over the 80 logits of each target's cell
            nc.scalar.activation(v3[:, :, 5:CH], v3[:, :, 5:CH], EXP)
            nc.scalar.activation(v3[:, :, 5:CH], v3[:, :, 5:CH], LN,
                                 bias=1.0,
                                 accum_out=part[:, OC_WSP:OC_WSP + 1])

            nc.scalar.dma_start(out=out_t.ap(), in_=part[:])
    bacc.get_activation_tables = _single_act_table
    try:
        nc.compile()
    finally:
        bacc.get_activation_tables = _ORIG_TABLES
    return nc


def get_nc():
    global _NC_CACHE
    if _NC_CACHE is None:
        _NC_CACHE = _build_nc()
    return _NC_CACHE


def prepare_in_maps(pred0, pred1, pred2, targets):
    """Host-side sharding + layout/index preprocessing (numpy only)."""
    preds = (np.asarray(pred0, dtype=np.float32),
             np.asarray(pred1, dtype=np.float32),
             np.asarray(pred2, dtype=np.float32))
    t = np.asarray(targets, dtype=np.float32)
    n = t.shape[0]
    b = t[:, 0].astype(np.int32)
    cls = t[:, 1].astype(np.int32)
    cx, cy, bw, bh = t[:, 2], t[:, 3], t[:, 4], t[:, 5]

    area = np.maximum(bw * bh, np.float32(1e-6))
    s_idx = np.where(area <= 0.01, 0,
                     np.where(area <= 0.03, 1, 2)).astype(np.int32)
    sw = np.float32(1.0) + STAL_GAMMA * (np.float32(1.0) - np.sqrt(area))

    ws = np.array(WS, np.int32)[s_idx]
    wf = ws.astype(np.float32)
    gx = np.clip((cx * wf).astype(np.int32), 0, ws - 1)
    gy = np.clip((cy * wf).astype(np.int32), 0, ws - 1)

    b_cl = np.clip(b, 0, BATCH - 1)
    core = b_cl // BPC

    valid_cls = ((cls >= 0) & (cls < NUM_CLASSES)).astype(np.float32)
    cls_c = np.clip(cls, 0, NUM_CLASSES - 1)

    # gather the 85-float pred row for every target (pure data movement)
    va_all = np.empty((n, CH), np.float32)
    for s in range(3):
        m = np.nonzero(s_idx == s)[0]
        if len(m):
            va_all[m] = preds[s][b_cl[m], :, gy[m], gx[m]]
    corr_all = va_all[np.arange(n), 5 + cls_c] * valid_cls

    # obj dedup: one representative target per (scale, batch, gy, gx) cell
    key = ((s_idx.astype(np.int64) * BATCH + b_cl) * 128 + gy) * 128 + gx
    dflag = np.zeros(n, np.float32)
    _, first = np.unique(key, return_index=True)
    dflag[first] = 1.0
    wobj_all = dflag / (np.float32(BATCH) * np.array(HW, np.float32)[s_idx])

    hw_denoms = np.array(HW, np.float32)
    in_maps = []
    for c in range(NCORES):
        sel = np.nonzero(core == c)[0]
        if len(sel) > TPAD:
            sel = sel[:TPAD]  # graceful degradation; never expected
        m = len(sel)

        # target t maps to (partition, group) = (t % 128, t // 128)
        def put_il(width, vals):  # [m, width] -> [128, GROUPS*width]
            buf = np.zeros((TPAD, width), np.float32)
            buf[:m] = vals
            return buf.reshape(GROUPS, 128, width).transpose(1, 0, 2).reshape(
                128, GROUPS * width)

        tgt = np.empty((128, NTGT), np.float32)
        va = np.full((TPAD, CH), PAD_VAL, np.float32)
        va[:m] = va_all[sel]
        tgt[:, TC_VA:TC_SUB] = va.reshape(GROUPS, 128, CH).transpose(
            1, 0, 2).reshape(128, GROUPS * CH)
        invw = np.float32(1.0) / wf[sel]
        tgt[:, TC_SUB:TC_SWM] = put_il(4, np.stack([
            cx[sel] * wf[sel] - gx[sel],
            cy[sel] * wf[sel] - gy[sel],
            bw[sel] * wf[sel],
            bh[sel] * wf[sel]], axis=1))
        tgt[:, TC_SWM:TC_WOB] = put_il(1, (sw[sel] * np.float32(0.25)
                                           * invw)[:, None])
        tgt[:, TC_WOB:TC_COR] = put_il(1, wobj_all[sel][:, None])
        tgt[:, TC_COR:NTGT] = put_il(1, corr_all[sel][:, None])

        lo, hi = c * BPC, (c + 1) * BPC
        obj = np.full((128, sum(OBJ_COLS)), np.float32(-100.0), np.float32)
        ocol = 0
        for s, p in enumerate(preds):
            nc_s = BPC * HW[s]
            w = OBJ_COLS[s]
            tmp = np.full(128 * w, np.float32(-100.0), np.float32)
            tmp[:nc_s] = p[lo:hi, 4].reshape(-1)
            obj[:, ocol:ocol + w] = tmp.reshape(128, w)
            ocol += w

        in_maps.append({
            "OBJ": obj.astype(BF16_NP),
            "TGT": tgt.astype(BF16_NP),
        })
    return in_maps, n


def finalize(results, n):
    """Combine per-core [128, NOUT] partial tiles into the 4 losses."""
    ps = np.stack([np.asarray(r["OUT"], np.float64) for r in results])
    cls_sp = ps[:, :, OC_WSP].sum()
    obj_sp = [ps[:, :, OC_OBJ + s].sum() for s in range(3)]
    box = ps[:, :, OC_BOX].sum()
    pos = ps[:, :, OC_POS].sum()
    corr = ps[:, :, OC_CORR].sum()

    norm = max(1, n)
    box_loss = box / norm
    cls_loss = (cls_sp - corr) / (NUM_CLASSES * norm)
    obj_loss = sum(obj_sp[s] / (BATCH * HW[s]) for s in range(3)) - pos
    total = box_loss + obj_loss + cls_loss
    return np.array([total, box_loss, obj_loss, cls_loss], np.float32)


def run_on_hw(in_maps, trace=False):
    nc = get_nc()
    return bass_utils.run_bass_kernel_spmd(
        nc, in_maps, core_ids=list(range(NCORES)), trace=trace)


def kernel(pred0, pred1, pred2, targets, **_unused):
    in_maps, n = prepare_in_maps(pred0, pred1, pred2, targets)
    res = run_on_hw(in_maps)
    return finalize(res.results, n)


# revision 15
# speedup vs baseline: 1.1699x; 1.1699x over previous
"""Trainium2 Bass kernel for a YOLO-style detection loss.

Sharding: data-parallel over batch — 8 NeuronCores, 4 batches/core.
Per-core partial sums land in a [128, 7] tile; the host sums the 8
tiles and assembles the 4 scalar losses (replacing the all-reduce of
4 scalars).

The loss touches pred densely only through the objectness channel
(BCE vs 0 over every cell); the class/box terms need the 85 logits at
the <=2048 assigned cells.  The host routes data (extracts channel 4,
gathers the 85-float rows per target, precomputes target-derived
constants: grid offsets, small_weight, dedup flags) — all pure data
movement/indexing, as in the indirect-DMA version, but the gather now
happens host-side so the device never pays the serialized GpSimd
indirect-DMA issue + latency chain.  All loss arithmetic on pred
values runs on device:

1. OBJ stream: softplus over channel 4 of every cell (one [128, 263]
   bf16 tile): Exp pass then per-scale Ln(1+x) passes whose accum_out
   gives the per-scale column sums directly.
2. TGT stream: one [128, 276] bf16 tile holding 3 groups x 85 gathered
   logits + per-target constants.  Box decode uses Exp(scale=-1) +
   add/reciprocal for sigmoid, Exp(min(x,4)) for wh; the weighted sums
   come out of scalar_tensor_tensor accum_out.  Class softplus sum
   falls out of the Ln pass's accum_out; the target-class logit
   correction is a host-gathered column the device just sums.

softplus(x) = ln(exp(x) + 1); Exp/Ln are pinned to the single ACT
table that holds both (natural_log_exp_and_others) to avoid
per-instruction table reloads.  bf16 tiles: all accumulations land in
fp32 (accum_out / reduce dst); tolerance is 2e-2, bf16 input rounding
contributes ~1e-4.
"""

import numpy as np
import ml_dtypes

from concourse import bass, bacc, mybir
from concourse import bass_utils
from concourse.tile import TileContext

F32 = mybir.dt.float32
BF16 = mybir.dt.bfloat16
BF16_NP = ml_dtypes.bfloat16

NUM_CLASSES = 80
STAL_GAMMA = np.float32(2.0)
BATCH = 32
NCORES = 8
BPC = BATCH // NCORES          # batches per core
CH = 5 + NUM_CLASSES
HW = (80 * 80, 40 * 40, 20 * 20)
WS = (80, 40, 20)
# OBJ stream: per-scale column blocks, scale 2 padded to 128*13
OBJ_COLS = (HW[0] * BPC // 128, HW[1] * BPC // 128, 1664 // 128)  # 200,50,13
GROUPS = 3                                  # 128 targets each
TPAD = 128 * GROUPS                         # 384; mean load is ~256/core
PAD_VAL = np.float32(-30.0)                 # neutral logit for padding rows
# TGT tile column layout (bf16), GROUPS-interleaved like the VA rows
TC_VA = 0                                   # 3 x 85 gathered logits
TC_SUB = GROUPS * CH                        # 255: w*cx-gx etc, 3 x 4
TC_SWM = TC_SUB + GROUPS * 4                # 267: sw/4/w              3
TC_WOB = TC_SWM + GROUPS                    # 270: dedup/(B*HW_s)      3
TC_COR = TC_WOB + GROUPS                    # 273: target-class logit  3
NTGT = TC_COR + GROUPS                      # 276
# output partial tile column layout
OC_WSP = 0      # class softplus-sum term
OC_OBJ = 1      # 3 cols: per-scale objectness softplus sums
OC_BOX = 4
OC_POS = 5      # objectness positive-cell correction (pre-scaled)
OC_CORR = 6
NOUT = 7

_NC_CACHE = None


def _single_act_table(arch):
    """Empty out every activation table except natural_log_exp_and_others
    (which holds all the functions this kernel uses), so the table-load
    pass can only ever pick that one table -> exactly one ACT_TABLE_LOAD
    instead of a conservative extra load of table 0."""
    tabs = _ORIG_TABLES(arch)
    out = {}
    for name, fns in tabs.items():
        out[name] = fns if name == "natural_log_exp_and_others" \
            else type(fns)()
    return out


_ORIG_TABLES = bacc.get_activation_tables


def _build_nc():
    nc = bacc.Bacc("TRN2", target_bir_lowering=False, debug=False)
    obj_t = nc.dram_tensor("OBJ", [128, sum(OBJ_COLS)], BF16,
                           kind="ExternalInput")
    tgt_t = nc.dram_tensor("TGT", [128, NTGT], BF16, kind="ExternalInput")
    out_t = nc.dram_tensor("OUT", [128, NOUT], F32, kind="ExternalOutput")

    EXP = mybir.ActivationFunctionType.Exp
    LN = mybir.ActivationFunctionType.Ln
    AX = mybir.AxisListType
    ALU = mybir.AluOpType
    NOB = sum(OBJ_COLS)
    with nc.allow_low_precision("bf16 validated: tolerance 2e-2, "
                                "bf16 rounding contributes ~1e-4"), \
            TileContext(nc) as tc:
        with tc.tile_pool(name="persist", bufs=1) as pp:
            part = pp.tile([128, NOUT], F32)
            tg = pp.tile([128, NTGT], BF16)
            ob = pp.tile([128, NOB], BF16)
            l1 = pp.tile([128, GROUPS], BF16)
            g3 = pp.tile([128, GROUPS], BF16)
            sc = pp.tile([128, GROUPS], BF16)

            # TGT on the scalar HWDGE ring, OBJ on the sync ring (the
            # only two hardware DGE rings); OUT reuses the warm scalar
            # ring at the end
            nc.scalar.dma_start(out=tg[:], in_=tgt_t.ap())
            nc.sync.dma_start(out=ob[:], in_=obj_t.ap())

            v3 = tg[:, :TC_SUB].rearrange("p (j c) -> p j c", c=CH)
            sub3 = tg[:, TC_SUB:TC_SWM].rearrange("p (j c) -> p j c", c=4)

            # ---- per-target math ----
            # negate-and-clamp wh so ONE Exp(scale=-1) pass decodes all
            # four box channels: exp(-(-min(x,4))) == exp(min(x,4))
            nc.vector.tensor_scalar(v3[:, :, 2:4], v3[:, :, 2:4], -1.0, -4.0,
                                    op0=ALU.mult, op1=ALU.max)
            nc.scalar.activation(v3[:, :, 0:4], v3[:, :, 0:4], EXP,
                                 scale=-1.0)
            # sigmoid = 1/(1+exp(-x)) via DVE reciprocal
            nc.vector.tensor_scalar_add(v3[:, :, 0:2], v3[:, :, 0:2], 1.0)
            nc.vector.reciprocal(v3[:, :, 0:2], v3[:, :, 0:2])
            # objectness positive-cell correction (raw channel 4),
            # per-scale 1/(B*HW_s) prefolded into the WOB column
            nc.vector.scalar_tensor_tensor(
                sc[:], v3[:, :, 4], 0.0, tg[:, TC_WOB:TC_WOB + GROUPS],
                op0=ALU.bypass, op1=ALU.mult,
                accum_out=part[:, OC_POS:OC_POS + 1])
            # class-logit correction: host-gathered column, just sum it
            nc.vector.reduce_sum(part[:, OC_CORR:OC_CORR + 1],
                                 tg[:, TC_COR:TC_COR + GROUPS], axis=AX.X)
            # ---- dense objectness stream (ACT engine) ----
            nc.scalar.activation(ob[:], ob[:], EXP)
            nc.scalar.activation(ob[:], ob[:], LN, bias=1.0)
            ocol = 0
            for s in range(3):
                w = OBJ_COLS[s]
                nc.vector.reduce_sum(part[:, OC_OBJ + s:OC_OBJ + s + 1],
                                     ob[:, ocol:ocol + w], axis=AX.X)
                ocol += w
            # l1 in grid units: |dec - (w*tgt - g)|; 1/w folded into SWM
            nc.vector.tensor_sub(v3[:, :, 0:4], v3[:, :, 0:4], sub3)
            nc.vector.reduce_sum(l1[:], v3[:, :, 0:4], axis=AX.X,
                                 apply_absolute_value=True)
            nc.vector.scalar_tensor_tensor(
                g3[:], l1[:], 0.0, tg[:, TC_SWM:TC_SWM + GROUPS],
                op0=ALU.bypass, op1=ALU.mult,
                accum_out=part[:, OC_BOX:OC_BOX + 1])
            # class softplus sum over the 80 logits of each target's cell
            nc.scalar.activation(v3[:, :, 5:CH], v3[:, :, 5:CH], EXP)
            nc.scalar.activation(v3[:, :, 5:CH], v3[:, :, 5:CH], LN,
                                 bias=1.0,
                                 accum_out=part[:, OC_WSP:OC_WSP + 1])

            nc.scalar.dma_start(out=out_t.ap(), in_=part[:])
    bacc.get_activation_tables = _single_act_table
    try:
        nc.compile()
    finally:
        bacc.get_activation_tables = _ORIG_TABLES
    _hoist_input_dmas(nc)
    return nc


def _hoist_input_dmas(nc):
    """Move the two input DMA issues (no waits, sem-update only) from the
    tile body block into the program entry block, ahead of the const
    memsets and the all-engine entry barrier.  The HWDGE doorbell +
    descriptor fetch + transfer then overlap the ~1us framework prologue
    instead of starting after it; consumers still wait on the DMAs'
    completion semaphores."""
    f = nc.m.functions[0]
    entry, body = f.blocks[0], f.blocks[1]
    hoist = [i for i in body.instructions
             if isinstance(i, mybir.InstDMACopy)
             and getattr(i.ins[0], "memref", None) in ("TGT", "OBJ")]
    assert len(hoist) == 2, [i.name for i in hoist]
    for i in hoist:
        assert not (i.sync_info and i.sync_info.on_wait)
        body.instructions.remove(i)
    entry.instructions[1:1] = hoist


def get_nc():
    global _NC_CACHE
    if _NC_CACHE is None:
        _NC_CACHE = _build_nc()
    return _NC_CACHE


def prepare_in_maps(pred0, pred1, pred2, targets):
    """Host-side sharding + layout/index preprocessing (numpy only)."""
    preds = (np.asarray(pred0, dtype=np.float32),
             np.asarray(pred1, dtype=np.float32),
             np.asarray(pred2, dtype=np.float32))
    t = np.asarray(targets, dtype=np.float32)
    n = t.shape[0]
    b = t[:, 0].astype(np.int32)
    cls = t[:, 1].astype(np.int32)
    cx, cy, bw, bh = t[:, 2], t[:, 3], t[:, 4], t[:, 5]

    area = np.maximum(bw * bh, np.float32(1e-6))
    s_idx = np.where(area <= 0.01, 0,
                     np.where(area <= 0.03, 1, 2)).astype(np.int32)
    sw = np.float32(1.0) + STAL_GAMMA * (np.float32(1.0) - np.sqrt(area))

    ws = np.array(WS, np.int32)[s_idx]
    wf = ws.astype(np.float32)
    gx = np.clip((cx * wf).astype(np.int32), 0, ws - 1)
    gy = np.clip((cy * wf).astype(np.int32), 0, ws - 1)

    b_cl = np.clip(b, 0, BATCH - 1)
    core = b_cl // BPC

    valid_cls = ((cls >= 0) & (cls < NUM_CLASSES)).astype(np.float32)
    cls_c = np.clip(cls, 0, NUM_CLASSES - 1)

    # gather the 85-float pred row for every target (pure data movement)
    va_all = np.empty((n, CH), np.float32)
    for s in range(3):
        m = np.nonzero(s_idx == s)[0]
        if len(m):
            va_all[m] = preds[s][b_cl[m], :, gy[m], gx[m]]
    corr_all = va_all[np.arange(n), 5 + cls_c] * valid_cls

    # obj dedup: one representative target per (scale, batch, gy, gx) cell
    key = ((s_idx.astype(np.int64) * BATCH + b_cl) * 128 + gy) * 128 + gx
    dflag = np.zeros(n, np.float32)
    _, first = np.unique(key, return_index=True)
    dflag[first] = 1.0
    wobj_all = dflag / (np.float32(BATCH) * np.array(HW, np.float32)[s_idx])

    hw_denoms = np.array(HW, np.float32)
    in_maps = []
    for c in range(NCORES):
        sel = np.nonzero(core == c)[0]
        if len(sel) > TPAD:
            sel = sel[:TPAD]  # graceful degradation; never expected
        m = len(sel)

        # target t maps to (partition, group) = (t % 128, t // 128)
        def put_il(width, vals):  # [m, width] -> [128, GROUPS*width]
            buf = np.zeros((TPAD, width), np.float32)
            buf[:m] = vals
            return buf.reshape(GROUPS, 128, width).transpose(1, 0, 2).reshape(
                128, GROUPS * width)

        tgt = np.empty((128, NTGT), np.float32)
        va = np.full((TPAD, CH), PAD_VAL, np.float32)
        va[:m] = va_all[sel]
        tgt[:, TC_VA:TC_SUB] = va.reshape(GROUPS, 128, CH).transpose(
            1, 0, 2).reshape(128, GROUPS * CH)
        invw = np.float32(1.0) / wf[sel]
        tgt[:, TC_SUB:TC_SWM] = put_il(4, np.stack([
            cx[sel] * wf[sel] - gx[sel],
            cy[sel] * wf[sel] - gy[sel],
            bw[sel] * wf[sel],
            bh[sel] * wf[sel]], axis=1))
        tgt[:, TC_SWM:TC_WOB] = put_il(1, (sw[sel] * np.float32(0.25)
                                           * invw)[:, None])
        tgt[:, TC_WOB:TC_COR] = put_il(1, wobj_all[sel][:, None])
        tgt[:, TC_COR:NTGT] = put_il(1, corr_all[sel][:, None])

        lo, hi = c * BPC, (c + 1) * BPC
        obj = np.full((128, sum(OBJ_COLS)), np.float32(-100.0), np.float32)
        ocol = 0
        for s, p in enumerate(preds):
            nc_s = BPC * HW[s]
            w = OBJ_COLS[s]
            tmp = np.full(128 * w, np.float32(-100.0), np.float32)
            tmp[:nc_s] = p[lo:hi, 4].reshape(-1)
            obj[:, ocol:ocol + w] = tmp.reshape(128, w)
            ocol += w

        in_maps.append({
            "OBJ": obj.astype(BF16_NP),
            "TGT": tgt.astype(BF16_NP),
        })
    return in_maps, n


def finalize(results, n):
    """Combine per-core [128, NOUT] partial tiles into the 4 losses."""
    ps = np.stack([np.asarray(r["OUT"], np.float64) for r in results])
    cls_sp = ps[:, :, OC_WSP].sum()
    obj_sp = [ps[:, :, OC_OBJ + s].sum() for s in range(3)]
    box = ps[:, :, OC_BOX].sum()
    pos = ps[:, :, OC_POS].sum()
    corr = ps[:, :, OC_CORR].sum()

    norm = max(1, n)
    box_loss = box / norm
    cls_loss = (cls_sp - corr) / (NUM_CLASSES * norm)
    obj_loss = sum(obj_sp[s] / (BATCH * HW[s]) for s in range(3)) - pos
    total = box_loss + obj_loss + cls_loss
    return np.array([total, box_loss, obj_loss, cls_loss], np.float32)


def run_on_hw(in_maps, trace=False):
    nc = get_nc()
    return bass_utils.run_bass_kernel_spmd(
        nc, in_maps, core_ids=list(range(NCORES)), trace=trace)


def kernel(pred0, pred1, pred2, targets, **_unused):
    in_maps, n = prepare_in_maps(pred0, pred1, pred2, targets)
    res = run_on_hw(in_maps)
    return finalize(res.results, n)


# revision 17
# speedup vs baseline: 1.2091x; 1.0335x over previous
"""Trainium2 Bass kernel for a YOLO-style detection loss.

Sharding: data-parallel over batch — 8 NeuronCores, 4 batches/core.
Per-core partial sums land in a [128, 7] tile; the host sums the 8
tiles and assembles the 4 scalar losses (replacing the all-reduce of
4 scalars).

The loss touches pred densely only through the objectness channel
(BCE vs 0 over every cell); the class/box terms need the 85 logits at
the <=2048 assigned cells.  The host routes data (extracts channel 4,
gathers the 85-float rows per target, precomputes target-derived
constants: grid offsets, small_weight, dedup flags) — all pure data
movement/indexing, as in the indirect-DMA version, but the gather now
happens host-side so the device never pays the serialized GpSimd
indirect-DMA issue + latency chain.  All loss arithmetic on pred
values runs on device:

1. OBJ stream: softplus over channel 4 of every cell (one [128, 263]
   bf16 tile): Exp pass then per-scale Ln(1+x) passes whose accum_out
   gives the per-scale column sums directly.
2. TGT stream: one [128, 276] bf16 tile holding 3 groups x 85 gathered
   logits + per-target constants.  Box decode uses Exp(scale=-1) +
   add/reciprocal for sigmoid, Exp(min(x,4)) for wh; the weighted sums
   come out of scalar_tensor_tensor accum_out.  Class softplus sum
   falls out of the Ln pass's accum_out; the target-class logit
   correction is a host-gathered column the device just sums.

softplus(x) = ln(exp(x) + 1); Exp/Ln are pinned to the single ACT
table that holds both (natural_log_exp_and_others) to avoid
per-instruction table reloads.  bf16 tiles: all accumulations land in
fp32 (accum_out / reduce dst); tolerance is 2e-2, bf16 input rounding
contributes ~1e-4.
"""

import numpy as np
import ml_dtypes

from concourse import bass, bacc, mybir
from concourse import bass_utils
from concourse.tile import TileContext

F32 = mybir.dt.float32
BF16 = mybir.dt.bfloat16
BF16_NP = ml_dtypes.bfloat16

NUM_CLASSES = 80
STAL_GAMMA = np.float32(2.0)
BATCH = 32
NCORES = 8
BPC = BATCH // NCORES          # batches per core
CH = 5 + NUM_CLASSES
HW = (80 * 80, 40 * 40, 20 * 20)
WS = (80, 40, 20)
# OBJ stream: per-scale column blocks, scale 2 padded to 128*13
OBJ_COLS = (HW[0] * BPC // 128, HW[1] * BPC // 128, 1664 // 128)  # 200,50,13
GROUPS = 3                                  # 128 targets each
TPAD = 128 * GROUPS                         # 384; mean load is ~256/core
PAD_VAL = np.float32(-30.0)                 # neutral logit for padding rows
# TGT tile column layout (bf16), GROUPS-interleaved like the VA rows
TC_VA = 0                                   # 3 x 85 gathered logits
TC_SUB = GROUPS * CH                        # 255: w*cx-gx etc, 3 x 4
TC_SWM = TC_SUB + GROUPS * 4                # 267: sw/4/w              3
TC_WOB = TC_SWM + GROUPS                    # 270: dedup/(B*HW_s)      3
TC_COR = TC_WOB + GROUPS                    # 273: target-class logit  3
NTGT = TC_COR + GROUPS                      # 276
# output partial tile column layout
OC_WSP = 0      # class softplus-sum term
OC_OBJ = 1      # 3 cols: per-scale objectness softplus sums
OC_BOX = 4
OC_POS = 5      # objectness positive-cell correction (pre-scaled)
OC_CORR = 6
NOUT = 7

_NC_CACHE = None


def _single_act_table(arch):
    """Empty out every activation table except natural_log_exp_and_others
    (which holds all the functions this kernel uses), so the table-load
    pass can only ever pick that one table -> exactly one ACT_TABLE_LOAD
    instead of a conservative extra load of table 0."""
    tabs = _ORIG_TABLES(arch)
    out = {}
    for name, fns in tabs.items():
        out[name] = fns if name == "natural_log_exp_and_others" \
            else type(fns)()
    return out


_ORIG_TABLES = bacc.get_activation_tables


def _build_nc():
    nc = bacc.Bacc("TRN2", target_bir_lowering=False, debug=False)
    obj_t = nc.dram_tensor("OBJ", [128, sum(OBJ_COLS)], BF16,
                           kind="ExternalInput")
    tgt_t = nc.dram_tensor("TGT", [128, NTGT], BF16, kind="ExternalInput")
    out_t = nc.dram_tensor("OUT", [128, NOUT], F32, kind="ExternalOutput")

    EXP = mybir.ActivationFunctionType.Exp
    LN = mybir.ActivationFunctionType.Ln
    AX = mybir.AxisListType
    ALU = mybir.AluOpType
    NOB = sum(OBJ_COLS)
    with nc.allow_low_precision("bf16 validated: tolerance 2e-2, "
                                "bf16 rounding contributes ~1e-4"), \
            TileContext(nc) as tc:
        with tc.tile_pool(name="persist", bufs=1) as pp:
            part = pp.tile([128, NOUT], F32)
            tg = pp.tile([128, NTGT], BF16)
            ob = pp.tile([128, NOB], BF16)
            l1 = pp.tile([128, GROUPS], BF16)
            g3 = pp.tile([128, GROUPS], BF16)
            sc = pp.tile([128, GROUPS], BF16)

            # TGT on the scalar HWDGE ring, OBJ on the sync ring (the
            # only two hardware DGE rings); OUT reuses the warm scalar
            # ring at the end
            nc.scalar.dma_start(out=tg[:], in_=tgt_t.ap())
            nc.sync.dma_start(out=ob[:], in_=obj_t.ap())

            v3 = tg[:, :TC_SUB].rearrange("p (j c) -> p j c", c=CH)
            sub3 = tg[:, TC_SUB:TC_SWM].rearrange("p (j c) -> p j c", c=4)

            # ---- per-target math ----
            # negate-and-clamp wh so ONE Exp(scale=-1) pass decodes all
            # four box channels: exp(-(-min(x,4))) == exp(min(x,4))
            nc.vector.tensor_scalar(v3[:, :, 2:4], v3[:, :, 2:4], -1.0, -4.0,
                                    op0=ALU.mult, op1=ALU.max)
            nc.scalar.activation(v3[:, :, 0:4], v3[:, :, 0:4], EXP,
                                 scale=-1.0)
            # sigmoid = 1/(1+exp(-x)) via DVE reciprocal
            nc.vector.tensor_scalar_add(v3[:, :, 0:2], v3[:, :, 0:2], 1.0)
            nc.vector.reciprocal(v3[:, :, 0:2], v3[:, :, 0:2])
            # objectness positive-cell correction (raw channel 4),
            # per-scale 1/(B*HW_s) prefolded into the WOB column
            nc.vector.scalar_tensor_tensor(
                sc[:], v3[:, :, 4], 0.0, tg[:, TC_WOB:TC_WOB + GROUPS],
                op0=ALU.bypass, op1=ALU.mult,
                accum_out=part[:, OC_POS:OC_POS + 1])
            # class-logit correction: host-gathered column, just sum it
            nc.vector.reduce_sum(part[:, OC_CORR:OC_CORR + 1],
                                 tg[:, TC_COR:TC_COR + GROUPS], axis=AX.X)
            # ---- dense objectness stream (ACT engine) ----
            nc.scalar.activation(ob[:], ob[:], EXP)
            nc.scalar.activation(ob[:], ob[:], LN, bias=1.0)
            ocol = 0
            for s in range(3):
                w = OBJ_COLS[s]
                nc.vector.reduce_sum(part[:, OC_OBJ + s:OC_OBJ + s + 1],
                                     ob[:, ocol:ocol + w], axis=AX.X)
                ocol += w
            # l1 in grid units: |dec - (w*tgt - g)|; 1/w folded into SWM
            nc.vector.tensor_sub(v3[:, :, 0:4], v3[:, :, 0:4], sub3)
            nc.vector.reduce_sum(l1[:], v3[:, :, 0:4], axis=AX.X,
                                 apply_absolute_value=True)
            nc.vector.scalar_tensor_tensor(
                g3[:], l1[:], 0.0, tg[:, TC_SWM:TC_SWM + GROUPS],
                op0=ALU.bypass, op1=ALU.mult,
                accum_out=part[:, OC_BOX:OC_BOX + 1])
            # class softplus sum over the 80 logits of each target's cell
            nc.scalar.activation(v3[:, :, 5:CH], v3[:, :, 5:CH], EXP)
            nc.scalar.activation(v3[:, :, 5:CH], v3[:, :, 5:CH], LN,
                                 bias=1.0,
                                 accum_out=part[:, OC_WSP:OC_WSP + 1])

            nc.sync.dma_start(out=out_t.ap(), in_=part[:])
    bacc.get_activation_tables = _single_act_table
    try:
        nc.compile()
    finally:
        bacc.get_activation_tables = _ORIG_TABLES
    _hoist_input_dmas(nc)
    return nc


def _hoist_input_dmas(nc):
    """Move the two input DMA issues (no waits, sem-update only) from the
    tile body block into the program entry block, ahead of the const
    memsets and the all-engine entry barrier.  The HWDGE doorbell +
    descriptor fetch + transfer then overlap the ~1us framework prologue
    instead of starting after it; consumers still wait on the DMAs'
    completion semaphores."""
    f = nc.m.functions[0]
    entry, body = f.blocks[0], f.blocks[1]
    hoist = [i for i in body.instructions
             if isinstance(i, mybir.InstDMACopy)
             and getattr(i.ins[0], "memref", None) in ("TGT", "OBJ")]
    assert len(hoist) == 2, [i.name for i in hoist]
    # the activation-table load is also dependency-free; issuing it right
    # after the TGT DMA overlaps it with the DMA doorbell+transfer
    tab = [i for i in body.instructions
           if isinstance(i, mybir.InstLoadActFuncSet)]
    assert len(tab) == 1
    hoist += tab
    for i in hoist:
        assert not (i.sync_info and i.sync_info.on_wait)
        body.instructions.remove(i)
    entry.instructions[1:1] = hoist


def get_nc():
    global _NC_CACHE
    if _NC_CACHE is None:
        _NC_CACHE = _build_nc()
    return _NC_CACHE


def prepare_in_maps(pred0, pred1, pred2, targets):
    """Host-side sharding + layout/index preprocessing (numpy only)."""
    preds = (np.asarray(pred0, dtype=np.float32),
             np.asarray(pred1, dtype=np.float32),
             np.asarray(pred2, dtype=np.float32))
    t = np.asarray(targets, dtype=np.float32)
    n = t.shape[0]
    b = t[:, 0].astype(np.int32)
    cls = t[:, 1].astype(np.int32)
    cx, cy, bw, bh = t[:, 2], t[:, 3], t[:, 4], t[:, 5]

    area = np.maximum(bw * bh, np.float32(1e-6))
    s_idx = np.where(area <= 0.01, 0,
                     np.where(area <= 0.03, 1, 2)).astype(np.int32)
    sw = np.float32(1.0) + STAL_GAMMA * (np.float32(1.0) - np.sqrt(area))

    ws = np.array(WS, np.int32)[s_idx]
    wf = ws.astype(np.float32)
    gx = np.clip((cx * wf).astype(np.int32), 0, ws - 1)
    gy = np.clip((cy * wf).astype(np.int32), 0, ws - 1)

    b_cl = np.clip(b, 0, BATCH - 1)
    core = b_cl // BPC

    valid_cls = ((cls >= 0) & (cls < NUM_CLASSES)).astype(np.float32)
    cls_c = np.clip(cls, 0, NUM_CLASSES - 1)

    # gather the 85-float pred row for every target (pure data movement)
    va_all = np.empty((n, CH), np.float32)
    for s in range(3):
        m = np.nonzero(s_idx == s)[0]
        if len(m):
            va_all[m] = preds[s][b_cl[m], :, gy[m], gx[m]]
    corr_all = va_all[np.arange(n), 5 + cls_c] * valid_cls

    # obj dedup: one representative target per (scale, batch, gy, gx) cell
    key = ((s_idx.astype(np.int64) * BATCH + b_cl) * 128 + gy) * 128 + gx
    dflag = np.zeros(n, np.float32)
    _, first = np.unique(key, return_index=True)
    dflag[first] = 1.0
    wobj_all = dflag / (np.float32(BATCH) * np.array(HW, np.float32)[s_idx])

    hw_denoms = np.array(HW, np.float32)
    in_maps = []
    for c in range(NCORES):
        sel = np.nonzero(core == c)[0]
        if len(sel) > TPAD:
            sel = sel[:TPAD]  # graceful degradation; never expected
        m = len(sel)

        # target t maps to (partition, group) = (t % 128, t // 128)
        def put_il(width, vals):  # [m, width] -> [128, GROUPS*width]
            buf = np.zeros((TPAD, width), np.float32)
            buf[:m] = vals
            return buf.reshape(GROUPS, 128, width).transpose(1, 0, 2).reshape(
                128, GROUPS * width)

        tgt = np.empty((128, NTGT), np.float32)
        va = np.full((TPAD, CH), PAD_VAL, np.float32)
        va[:m] = va_all[sel]
        tgt[:, TC_VA:TC_SUB] = va.reshape(GROUPS, 128, CH).transpose(
            1, 0, 2).reshape(128, GROUPS * CH)
        invw = np.float32(1.0) / wf[sel]
        tgt[:, TC_SUB:TC_SWM] = put_il(4, np.stack([
            cx[sel] * wf[sel] - gx[sel],
            cy[sel] * wf[sel] - gy[sel],
            bw[sel] * wf[sel],
            bh[sel] * wf[sel]], axis=1))
        tgt[:, TC_SWM:TC_WOB] = put_il(1, (sw[sel] * np.float32(0.25)
                                           * invw)[:, None])
        tgt[:, TC_WOB:TC_COR] = put_il(1, wobj_all[sel][:, None])
        tgt[:, TC_COR:NTGT] = put_il(1, corr_all[sel][:, None])

        lo, hi = c * BPC, (c + 1) * BPC
        obj = np.full((128, sum(OBJ_COLS)), np.float32(-100.0), np.float32)
        ocol = 0
        for s, p in enumerate(preds):
            nc_s = BPC * HW[s]
            w = OBJ_COLS[s]
            tmp = np.full(128 * w, np.float32(-100.0), np.float32)
            tmp[:nc_s] = p[lo:hi, 4].reshape(-1)
            obj[:, ocol:ocol + w] = tmp.reshape(128, w)
            ocol += w

        in_maps.append({
            "OBJ": obj.astype(BF16_NP),
            "TGT": tgt.astype(BF16_NP),
        })
    return in_maps, n


def finalize(results, n):
    """Combine per-core [128, NOUT] partial tiles into the 4 losses."""
    ps = np.stack([np.asarray(r["OUT"], np.float64) for r in results])
    cls_sp = ps[:, :, OC_WSP].sum()
    obj_sp = [ps[:, :, OC_OBJ + s].sum() for s in range(3)]
    box = ps[:, :, OC_BOX].sum()
    pos = ps[:, :, OC_POS].sum()
    corr = ps[:, :, OC_CORR].sum()

    norm = max(1, n)
    box_loss = box / norm
    cls_loss = (cls_sp - corr) / (NUM_CLASSES * norm)
    obj_loss = sum(obj_sp[s] / (BATCH * HW[s]) for s in range(3)) - pos
    total = box_loss + obj_loss + cls_loss
    return np.array([total, box_loss, obj_loss, cls_loss], np.float32)


def run_on_hw(in_maps, trace=False):
    nc = get_nc()
    return bass_utils.run_bass_kernel_spmd(
        nc, in_maps, core_ids=list(range(NCORES)), trace=trace)


def kernel(pred0, pred1, pred2, targets, **_unused):
    in_maps, n = prepare_in_maps(pred0, pred1, pred2, targets)
    res = run_on_hw(in_maps)
    return finalize(res.results, n)
